# revision 3
# baseline (speedup 1.0000x reference)
"""DiffusionStep kernel v3: gather-only design, no scatter.

Host assigns each node's edges round-robin over 8 cores (<=16 per
(node, core)) and lays each core's edge stream out in node-slot order:
node n owns columns [16*(n>>3), 16*(n>>3)+16) of partition-group
g = n % 8.  Device: ap_gather (d=2, 32-node blocks) fetches x
candidates, DVE mask (dst%32 == 2*(p%16)+s) selects, ones-matmul
contracts the 16 rows of each group -> per-edge messages at node-sorted
(g, column) positions, multiply by w, tensor_reduce 16->1 -> per-core
partials [8, 12544], AllReduce, affine relayout to out[N, 1].
"""
import sys

sys.path.insert(0, "/opt/trn_rl_repo")

import numpy as np

N = 100000
E = 6400000
NCORES = 8
G = 8
CH = 2048                  # columns (edges per Q7 core) per chunk
NCH_FULL = 98              # chunks: 98*2048 = 200704 columns per group
J = CH * NCH_FULL          # padded column count per partition-group
NPG = 12544                # node slots per group (= J / 16)
NE = 3125                  # ap_gather num_elems (32-node blocks)

_cache = {}


def _build(nch, passes=1):
    from concourse import bacc, mybir, tile

    nc = bacc.Bacc(None, target_bir_lowering=False)
    f32, i16, i8 = mybir.dt.float32, mybir.dt.int16, mybir.dt.int8

    j = CH * nch
    npg = j // 16
    dstw = nc.declare_dram_parameter("dstw", [128, j // 16], i16, isOutput=False)
    dlow = nc.declare_dram_parameter("dlow", [G, j], i8, isOutput=False)
    wst = nc.declare_dram_parameter("wst", [G, j], f32, isOutput=False)
    xtab = nc.declare_dram_parameter("xtab", [128, 2 * NE], f32, isOutput=False)
    iota2 = nc.declare_dram_parameter("iota2", [128, 2], f32, isOutput=False)
    bones = nc.declare_dram_parameter("bones", [128, G], f32, isOutput=False)
    out = nc.declare_dram_parameter("out", [N, 1], f32, isOutput=True)

    partial = nc.dram_tensor("partial", [G, npg], f32)
    reduced = nc.dram_tensor("reduced", [G, npg], f32, addr_space="Shared")

    with tile.TileContext(nc) as tc:
        with tc.tile_pool(name="const", bufs=1) as cpool, \
             tc.tile_pool(name="work", bufs=2) as pool, \
             tc.tile_pool(name="acc", bufs=1) as apool, \
             tc.tile_pool(name="ps", bufs=2, space="PSUM") as pspool:

            xtab_sb = cpool.tile([128, 2 * NE], f32, tag="xtab")
            nc.sync.dma_start(out=xtab_sb[:], in_=xtab[:])
            iota2_sb = cpool.tile([128, 2], f32, tag="iota2")
            nc.sync.dma_start(out=iota2_sb[:], in_=iota2[:])
            bones_sb = cpool.tile([128, G], f32, tag="bones")
            nc.sync.dma_start(out=bones_sb[:], in_=bones[:])

            outacc = apool.tile([G, npg], f32, tag="outacc")

            with tc.For_i(0, passes):
                for ch in range(nch):
                    csl = slice(ch * CH, (ch + 1) * CH)
                    wsl = slice(ch * (CH // 16), (ch + 1) * (CH // 16))

                    # gather 2 candidates per slot from the edge's 32-block
                    idx_c = pool.tile([128, CH // 16], i16, tag="idx_c")
                    nc.sync.dma_start(out=idx_c[:], in_=dstw[:, wsl])
                    cand = pool.tile([128, 2 * CH], f32, tag="cand")
                    nc.gpsimd.ap_gather(
                        out_ap=cand[:], in_ap=xtab_sb[:], idxs_ap=idx_c[:],
                        channels=128, num_elems=NE, d=2, num_idxs=CH,
                    )

                    # mask: keep candidate s at row p iff dst%32 == 2*(p%16)+s
                    dl8 = pool.tile([128, CH], i8, tag="dl8")
                    nc.sync.dma_start(
                        out=dl8[:],
                        in_=dlow[:, csl].unsqueeze(1).to_broadcast([G, 16, CH]),
                    )
                    dlf = pool.tile([128, CH], f32, tag="dlf")
                    nc.vector.tensor_copy(out=dlf[:], in_=dl8[:])
                    mask = pool.tile([128, 2 * CH], f32, tag="mask")
                    nc.vector.tensor_tensor(
                        out=mask[:].rearrange("p (j s) -> p j s", s=2),
                        in0=dlf[:].unsqueeze(2).to_broadcast([128, CH, 2]),
                        in1=iota2_sb[:].unsqueeze(1).to_broadcast([128, CH, 2]),
                        op=mybir.AluOpType.is_equal,
                    )
                    nc.vector.tensor_tensor(
                        out=cand[:], in0=cand[:], in1=mask[:],
                        op=mybir.AluOpType.mult,
                    )

                    # contract 16 rows per group -> per-edge messages [G, 2CH]
                    sb8 = pool.tile([G, CH], f32, tag="sb8")
                    for k in range(2 * CH // 512):
                        ps = pspool.tile([G, 512], f32, tag="ps")
                        nc.tensor.matmul(
                            out=ps[:], lhsT=bones_sb[:],
                            rhs=cand[:, k * 512:(k + 1) * 512],
                            start=True, stop=True,
                        )
                        nc.vector.tensor_reduce(
                            out=sb8[:, k * 256:(k + 1) * 256],
                            in_=ps[:].rearrange("g (j s) -> g j s", s=2),
                            axis=mybir.AxisListType.X, op=mybir.AluOpType.add,
                        )

                    # weights, then 16 -> 1 per node
                    w_c = pool.tile([G, CH], f32, tag="w_c")
                    nc.sync.dma_start(out=w_c[:], in_=wst[:, csl])
                    nc.vector.tensor_tensor(
                        out=sb8[:], in0=sb8[:], in1=w_c[:],
                        op=mybir.AluOpType.mult,
                    )
                    nc.vector.tensor_reduce(
                        out=outacc[:, ch * (CH // 16):(ch + 1) * (CH // 16)],
                        in_=sb8[:].rearrange("g (q i) -> g q i", i=16),
                        axis=mybir.AxisListType.X, op=mybir.AluOpType.add,
                    )

            # combine across cores
            nc.sync.dma_start(out=partial[:], in_=outacc[:])
            nc.gpsimd.collective_compute(
                "AllReduce",
                mybir.AluOpType.add,
                replica_groups=[list(range(NCORES))],
                ins=[partial[:]],
                outs=[reduced[:]],
            )
            # out[n] = reduced[n % 8, n >> 3]
            with nc.allow_non_contiguous_dma(reason="final relayout"):
                nc.sync.dma_start(
                    out=out[0:N, 0].rearrange("(q g) -> g q", g=G),
                    in_=reduced[:, 0:N // G],
                )

    nc.finalize()
    return nc


def _get_nc(nch):
    if nch not in _cache:
        _cache[nch] = _build(nch)
    return _cache[nch]


def _build_timing(nch, passes=2):
    key = (nch, passes)
    if key not in _cache:
        _cache[key] = _build(nch, passes=passes)
    return _cache[key]


def _host_prep(x, edge_index, edge_weight, nch):
    x = np.asarray(x, dtype=np.float32).reshape(N)
    ei = np.asarray(edge_index)
    src = ei[0].astype(np.int64)
    dst = ei[1].astype(np.int32)
    w = np.asarray(edge_weight, dtype=np.float32)
    j = CH * nch

    # sort by src; rank within node; round-robin cores; node-slot columns
    order = np.argsort(src, kind="stable")
    s_s, d_s, w_s = src[order], dst[order], w[order]
    first = np.zeros(N + 1, np.int64)
    np.add.at(first, s_s + 1, 1)
    starts = np.cumsum(first)[:-1]
    rank = np.arange(E, dtype=np.int64) - starts[s_s]
    core = (rank % NCORES).astype(np.int32)
    i16r = rank // NCORES
    assert i16r.max() < 16, f"per-(node,core) overflow: {i16r.max()}"
    g = (s_s % G).astype(np.int32)
    col = (s_s >> 3) * 16 + i16r
    assert col.max() < j

    xtab = np.tile(
        np.ascontiguousarray(
            x.reshape(NE, 16, 2).transpose(1, 0, 2).reshape(16, 2 * NE)
        ),
        (8, 1),
    )
    iota2 = np.stack(
        [2 * (np.arange(128) % 16), 2 * (np.arange(128) % 16) + 1], axis=1
    ).astype(np.float32)
    bones = np.zeros((128, G), np.float32)
    for gg in range(G):
        bones[16 * gg:16 * (gg + 1), gg] = 1.0

    in_maps = []
    for c in range(NCORES):
        m = core == c
        dc = np.zeros((G, j), np.int32)
        wc = np.zeros((G, j), np.float32)
        dc[g[m], col[m]] = d_s[m]
        wc[g[m], col[m]] = w_s[m]
        idx = (dc >> 5).astype(np.int16)            # [G, j]
        # wrap: dstw[16*gg + jj%16, jj//16] = idx[gg, jj]
        dstw = np.ascontiguousarray(
            idx.reshape(G, j // 16, 16).transpose(0, 2, 1)
        ).reshape(128, j // 16)
        in_maps.append({
            "dstw": dstw,
            "dlow": np.ascontiguousarray((dc & 31).astype(np.int8)),
            "wst": np.ascontiguousarray(wc),
            "xtab": xtab,
            "iota2": iota2,
            "bones": bones,
        })
    return in_maps


def kernel(x, edge_index, edge_weight, nch=NCH_FULL):
    from concourse.bass_utils import run_bass_kernel_spmd

    nc = _get_nc(nch)
    in_maps = _host_prep(x, edge_index, edge_weight, nch)
    res = run_bass_kernel_spmd(nc, in_maps, list(range(NCORES)))
    out = res.results[0]["out"].astype(np.float32).reshape(N, 1)
    return out
